# revision 7
# baseline (speedup 1.0000x reference)
"""Trainium2 Bass kernel for nn_MultiHeadMLP (GAT-style message passing).

Strategy (8 NeuronCores, SPMD):
  - Host shards edges by target-node range (12500 nodes per core) and sorts
    each shard by target; every per-edge array is a pure index-derived
    relayout of the inputs.
  - Each core builds the full node table [h | s_r] (redundantly), its own
    s_c table, then streams its edge shard: h[r] rows arrive via indirect
    DMA; the per-target one-hot matrices (built with is_equal against an
    iota) turn the segment softmax numerator/denominator sums into PE
    matmuls accumulated per 128-node window in PSUM.
  - Softmax max-subtraction is skipped: alpha magnitudes here are < 1, so
    exp() is numerically safe and the result is mathematically identical.
  - Output: normalize, add bias, multiply by W_out^T, add b_out; the host
    concatenates the 8 shards.
"""
import numpy as np
import os
import sys

if "/opt/trn_rl_repo" not in sys.path:
    sys.path.insert(0, "/opt/trn_rl_repo")

import concourse.bass as bass
import concourse.mybir as mybir
import concourse.tile as tile
from concourse.bass_utils import run_bass_kernel_spmd
from concourse.masks import make_identity

F32 = mybir.dt.float32
I32 = mybir.dt.int32

N = 100000
D = 64
H = 8
HD = 8
ED = 32
EHD = 4
NCORES = 8
NSH = N // NCORES            # 12500
NSH_PAD = 12544              # 98 * 128
NWIN = NSH_PAD // 128        # 98
N_PAD = 100352               # 784 * 128
PAD_CREL = 999.0


# ---------------------------------------------------------------------------
# walrus workaround: this build rejects instructions with >1 semaphore wait
def _split_multi_waits(nc):
    def _split_block(bb):
        insts = list(bb.instructions)
        out = []
        changed = False
        for inst in insts:
            si = inst.sync_info
            if si is not None:
                waits = list(si.on_wait)
                if len(waits) > 1:
                    for w in waits[:-1]:
                        out.append(mybir.InstNoOp(
                            name=nc.get_next_instruction_name(),
                            engine=inst.engine, ins=[], outs=[],
                            sync_info=mybir.SyncInfo(on_wait=[w], on_update=[]),
                        ))
                    inst.sync_info = mybir.SyncInfo(
                        on_wait=[waits[-1]], on_update=list(si.on_update))
                    changed = True
            out.append(inst)
        if changed:
            bb.instructions = out

    def _walk(blocks):
        for b in blocks:
            if hasattr(b, "instructions"):
                _split_block(b)
            if hasattr(b, "blocks"):
                _walk(b.blocks)

    for f in nc.m.functions:
        _walk(f.blocks)


# ---------------------------------------------------------------------------
def _host_prep(feats, edge_index, edge_attr):
    """Shard by target range, sort by target, build device layouts."""
    r = np.asarray(edge_index[:, 0]).astype(np.int64)
    c = np.asarray(edge_index[:, 1]).astype(np.int64)
    ea = np.asarray(edge_attr, np.float32)
    feats = np.asarray(feats, np.float32)
    feats_pad = np.zeros((N_PAD, D), np.float32)
    feats_pad[:N] = feats

    shards = []
    tiles_w_all = np.zeros((NCORES, NWIN), np.int64)
    for k in range(NCORES):
        sel = np.nonzero((c // NSH) == k)[0]
        c_k = c[sel] - k * NSH
        order = np.argsort(c_k, kind="stable")
        r_s = r[sel][order]
        c_s = c_k[order]
        ea_s = ea[sel][order]
        cnt = np.bincount(c_s // 128, minlength=NWIN)
        tiles_w_all[k] = (cnt + 127) // 128
        shards.append((r_s, c_s, ea_s, cnt))

    tiles_w = tiles_w_all.max(axis=0)          # common SPMD schedule
    win_start = np.zeros(NWIN + 1, np.int64)
    np.cumsum(tiles_w, out=win_start[1:])
    T = int(win_start[-1])
    Epad = T * 128

    cores = []
    for k in range(NCORES):
        r_s, c_s, ea_s, cnt = shards[k]
        r_pad = np.zeros(Epad, np.int32)
        crel_pad = np.full(Epad, PAD_CREL, np.float32)
        ea_pad = np.zeros((Epad, ED), np.float32)
        src = 0
        for wi in range(NWIN):
            n_e = int(cnt[wi])
            pos = int(win_start[wi]) * 128
            if n_e:
                r_pad[pos:pos + n_e] = r_s[src:src + n_e]
                crel_pad[pos:pos + n_e] = (c_s[src:src + n_e] - wi * 128)
                ea_pad[pos:pos + n_e] = ea_s[src:src + n_e]
            src += n_e
        cores.append(dict(
            r_lay=np.ascontiguousarray(r_pad.reshape(T, 128).T),      # [128,T] i32
            crel_lay=np.ascontiguousarray(crel_pad.reshape(T, 128).T),  # [128,T] f32
            crelT_lay=crel_pad.reshape(1, Epad).copy(),               # [1,128T] f32
            eaT_lay=np.ascontiguousarray(ea_pad.T),                   # [32,128T] f32
            feats_own=feats_pad[k * NSH:k * NSH + NSH_PAD].copy(),
        ))
    return feats_pad, cores, tiles_w, win_start, T


def _expand_att(att):
    """Block-diagonal relayout of att (values copied, no arithmetic)."""
    att = np.asarray(att, np.float32)[:, :, 0]            # [H, 2HD+EHD]
    bd_c = np.zeros((D, H), np.float32)
    bd_r = np.zeros((D, H), np.float32)
    bd_e = np.zeros((ED, H), np.float32)
    for h in range(H):
        bd_c[h * HD:(h + 1) * HD, h] = att[h, :HD]
        bd_r[h * HD:(h + 1) * HD, h] = att[h, HD:2 * HD]
        bd_e[h * EHD:(h + 1) * EHD, h] = att[h, 2 * HD:]
    return bd_c, bd_r, bd_e


# ---------------------------------------------------------------------------
def _build_nc(tiles_w, win_start, T):
    nc = bass.Bass()
    Epad = T * 128

    feats_d = nc.dram_tensor("feats_pad", [N_PAD, D], F32, kind="ExternalInput")
    fown_d = nc.dram_tensor("feats_own", [NSH_PAD, D], F32, kind="ExternalInput")
    wfc_d = nc.dram_tensor("W_fc", [D, D], F32, kind="ExternalInput")
    bfc_d = nc.dram_tensor("b_fc_col", [D, 1], F32, kind="ExternalInput")
    wedge_d = nc.dram_tensor("W_edge", [ED, ED], F32, kind="ExternalInput")
    bd_c_d = nc.dram_tensor("att_bd_c", [D, H], F32, kind="ExternalInput")
    bd_r_d = nc.dram_tensor("att_bd_r", [D, H], F32, kind="ExternalInput")
    bd_e_d = nc.dram_tensor("att_bd_e", [ED, H], F32, kind="ExternalInput")
    bias_d = nc.dram_tensor("bias_row", [1, D], F32, kind="ExternalInput")
    wout_d = nc.dram_tensor("W_out", [D, D], F32, kind="ExternalInput")
    bout_d = nc.dram_tensor("b_out_row", [1, D], F32, kind="ExternalInput")
    rlay_d = nc.dram_tensor("r_lay", [128, T], I32, kind="ExternalInput")
    crel_d = nc.dram_tensor("crel_lay", [128, T], F32, kind="ExternalInput")
    crelT_d = nc.dram_tensor("crelT_lay", [1, Epad], F32, kind="ExternalInput")
    eaT_d = nc.dram_tensor("eaT_lay", [ED, Epad], F32, kind="ExternalInput")

    ttab_d = nc.dram_tensor("T_tab", [N_PAD, 72], F32)     # [h | s_r]
    out_d = nc.dram_tensor("out_shard", [NSH_PAD, D], F32, kind="ExternalOutput")

    with tile.TileContext(nc) as tc:
        with tc.tile_pool(name="const", bufs=1) as cp, \
             tc.tile_pool(name="work", bufs=3) as wp, \
             tc.tile_pool(name="hgp", bufs=2) as hgp, \
             tc.tile_pool(name="ps", bufs=2, space="PSUM") as ps:

            # ---------------- P0: constants & small weights ----------------
            ident = cp.tile([128, 128], F32)
            make_identity(nc, ident[:])
            iota_i = cp.tile([128, 128], I32, tag="ioi")
            nc.gpsimd.iota(iota_i[:], pattern=[[1, 128]], channel_multiplier=0)
            iota_free = cp.tile([128, 128], F32)           # each row = 0..127
            nc.vector.tensor_copy(out=iota_free[:], in_=iota_i[:])
            iota_ci = cp.tile([128, 1], I32, tag="ioc")
            nc.gpsimd.iota(iota_ci[:], pattern=[[0, 1]], channel_multiplier=1)
            iota_col = cp.tile([128, 1], F32)              # partition index
            nc.vector.tensor_copy(out=iota_col[:], in_=iota_ci[:])
            ones_row = cp.tile([1, 128], F32)
            nc.vector.memset(ones_row[:], 1.0)

            wfc = cp.tile([D, D], F32)
            nc.sync.dma_start(out=wfc[:], in_=wfc_d[:])
            wedge = cp.tile([ED, ED], F32)
            nc.sync.dma_start(out=wedge[:], in_=wedge_d[:])
            bdc = cp.tile([D, H], F32)
            nc.sync.dma_start(out=bdc[:], in_=bd_c_d[:])
            bdr = cp.tile([D, H], F32)
            nc.sync.dma_start(out=bdr[:], in_=bd_r_d[:])
            bde = cp.tile([ED, H], F32)
            nc.sync.dma_start(out=bde[:], in_=bd_e_d[:])
            bfc_col = cp.tile([D, 1], F32)
            nc.sync.dma_start(out=bfc_col[:], in_=bfc_d[:])
            wout = cp.tile([D, D], F32)
            nc.sync.dma_start(out=wout[:], in_=wout_d[:])

            # R1 = [Wfc^T | B_r]  (rhs for the node-table matmul)
            r1_ps = ps.tile([D, 72], F32, tag="pmed")
            nc.tensor.transpose(out=r1_ps[:, 0:D], in_=wfc[:], identity=ident[0:D, 0:D])
            nc.tensor.matmul(out=r1_ps[:, D:72], lhsT=wfc[:], rhs=bdr[:],
                             start=True, stop=True)
            r1 = cp.tile([D, 72], F32)
            nc.vector.tensor_copy(out=r1[:], in_=r1_ps[:])
            # B_c, A_edge, W_out^T
            r2_ps = ps.tile([D, H], F32, tag="pmed")
            nc.tensor.matmul(out=r2_ps[:], lhsT=wfc[:], rhs=bdc[:], start=True, stop=True)
            r2 = cp.tile([D, H], F32)
            nc.vector.tensor_copy(out=r2[:], in_=r2_ps[:])
            ae_ps = ps.tile([ED, H], F32, tag="pmed")
            nc.tensor.matmul(out=ae_ps[:], lhsT=wedge[:], rhs=bde[:], start=True, stop=True)
            a_sb = cp.tile([ED, H], F32)
            nc.vector.tensor_copy(out=a_sb[:], in_=ae_ps[:])
            woutT_ps = ps.tile([D, D], F32, tag="pmed")
            nc.tensor.transpose(out=woutT_ps[:], in_=wout[:], identity=ident[0:D, 0:D])
            woutT = cp.tile([D, D], F32)
            nc.vector.tensor_copy(out=woutT[:], in_=woutT_ps[:])

            # bias rows: [b_fc | b_r_const] and b_c_const
            brow_ps = ps.tile([1, 72 + H], F32, tag="pmed")
            nc.tensor.matmul(out=brow_ps[:, D:72], lhsT=bfc_col[:], rhs=bdr[:],
                             start=True, stop=True)
            nc.tensor.matmul(out=brow_ps[:, 72:72 + H], lhsT=bfc_col[:], rhs=bdc[:],
                             start=True, stop=True)
            brow = cp.tile([1, 72 + H], F32)    # [:64]=b_fc, [64:72]=b_r, [72:80]=b_c
            nc.sync.dma_start(out=brow[0:1, 0:D], in_=bfc_d[:].rearrange("a b -> b a"))
            nc.vector.tensor_copy(out=brow[:, D:72 + H], in_=brow_ps[:, D:72 + H])
            # replicate to 128 partitions
            brep_ps = ps.tile([128, 72 + H + 2 * D], F32, tag="pmed")
            nc.tensor.matmul(out=brep_ps[:, :72 + H], lhsT=ones_row[:], rhs=brow[:],
                             start=True, stop=True)
            bias_row = cp.tile([1, D], F32)
            nc.sync.dma_start(out=bias_row[:], in_=bias_d[:])
            bout_row = cp.tile([1, D], F32)
            nc.sync.dma_start(out=bout_row[:], in_=bout_d[:])
            nc.tensor.matmul(out=brep_ps[:, 72 + H:72 + H + D], lhsT=ones_row[:],
                             rhs=bias_row[:], start=True, stop=True)
            nc.tensor.matmul(out=brep_ps[:, 72 + H + D:], lhsT=ones_row[:],
                             rhs=bout_row[:], start=True, stop=True)
            brep = cp.tile([128, 72 + H + 2 * D], F32)
            nc.vector.tensor_copy(out=brep[:], in_=brep_ps[:])
            # views
            bfc72_rep = brep[:, 0:72]       # node-table bias [b_fc | b_r]
            bc_rep = brep[:, 72:72 + H]
            bias_rep = brep[:, 72 + H:72 + H + D]
            bout_rep = brep[:, 72 + H + D:72 + H + 2 * D]

            # ---------------- P1: global node table T = [h | s_r] ----------
            NG = N_PAD // 512
            for g in range(NG):
                f4 = wp.tile([128, 4, D], F32, tag="f4")
                nc.sync.dma_start(
                    out=f4[:],
                    in_=feats_d[:].rearrange("(g j p) d -> g p j d", j=4, p=128)[g])
                ft_ps = ps.tile([D, 512], F32, tag="pbig")
                for j in range(4):
                    nc.tensor.transpose(out=ft_ps[:, j * 128:(j + 1) * 128],
                                        in_=f4[:, j, :], identity=ident[:])
                ft = wp.tile([D, 512], F32, tag="ft")
                nc.vector.tensor_copy(out=ft[:], in_=ft_ps[:])
                o_ps = ps.tile([128, 4, 72], F32, tag="pmed")
                for j in range(4):
                    nc.tensor.matmul(out=o_ps[:, j, :], lhsT=ft[:, j * 128:(j + 1) * 128],
                                     rhs=r1[:], start=True, stop=True)
                o_sb = wp.tile([128, 4, 72], F32, tag="osb")
                nc.vector.tensor_tensor(
                    out=o_sb[:], in0=o_ps[:],
                    in1=bfc72_rep.rearrange("p (x d) -> p x d", x=1).to_broadcast([128, 4, 72]),
                    op=mybir.AluOpType.add)
                nc.sync.dma_start(
                    out=ttab_d[:].rearrange("(g j p) d -> g p j d", j=4, p=128)[g],
                    in_=o_sb[:])

            # ---------------- P2: own s_c table (SBUF-resident) ------------
            sc_sb = cp.tile([128, NWIN, H], F32)
            NG2 = NSH_PAD // 256
            for g in range(NG2):
                f4 = wp.tile([128, 2, D], F32, tag="f4b")
                nc.sync.dma_start(
                    out=f4[:],
                    in_=fown_d[:].rearrange("(g j p) d -> g p j d", j=2, p=128)[g])
                ft_ps = ps.tile([D, 256], F32, tag="pbig")
                for j in range(2):
                    nc.tensor.transpose(out=ft_ps[:, j * 128:(j + 1) * 128],
                                        in_=f4[:, j, :], identity=ident[:])
                ft = wp.tile([D, 256], F32, tag="ft2")
                nc.vector.tensor_copy(out=ft[:], in_=ft_ps[:])
                s_ps = ps.tile([128, 2, H], F32, tag="pmed")
                for j in range(2):
                    nc.tensor.matmul(out=s_ps[:, j, :], lhsT=ft[:, j * 128:(j + 1) * 128],
                                     rhs=r2[:], start=True, stop=True)
                nc.vector.tensor_tensor(
                    out=sc_sb[:, 2 * g:2 * g + 2, :], in0=s_ps[:],
                    in1=bc_rep.rearrange("p (x d) -> p x d", x=1).to_broadcast([128, 2, H]),
                    op=mybir.AluOpType.add)

            # ---------------- P3: edge phase -------------------------------
            ridx = cp.tile([128, T], I32)
            nc.sync.dma_start(out=ridx[:], in_=rlay_d[:])
            crel = cp.tile([128, T], F32)
            nc.sync.dma_start(out=crel[:], in_=crel_d[:])
            tagg = cp.tile([128, NWIN, 72], F32)

            MAXTW = int(tiles_w.max())
            for w in range(NWIN):
                tw = int(tiles_w[w])
                t0 = int(win_start[w])
                if tw == 0:
                    continue
                crelT_w = wp.tile([1, MAXTW * 128], F32, tag="crelTw")
                nc.sync.dma_start(out=crelT_w[0:1, :tw * 128],
                                  in_=crelT_d[0:1, t0 * 128:(t0 + tw) * 128])
                eaT_w = wp.tile([ED, MAXTW * 128], F32, tag="eaTw")
                nc.sync.dma_start(out=eaT_w[:, :tw * 128],
                                  in_=eaT_d[:, t0 * 128:(t0 + tw) * 128])
                hg = hgp.tile([128, MAXTW, 72], F32, tag="hg")
                pw = ps.tile([128, 2 * MAXTW * H + 72], F32, tag="pswin")
                pay = hgp.tile([128, MAXTW, 72], F32, tag="pay")
                al = wp.tile([128, MAXTW, H], F32, tag="al")
                o_tiles = wp.tile([128, MAXTW, 128], F32, tag="otl")

                for tt in range(tw):
                    t = t0 + tt
                    nc.gpsimd.indirect_dma_start(
                        out=hg[:, tt, :], out_offset=None, in_=ttab_d[:],
                        in_offset=bass.IndirectOffsetOnAxis(ap=ridx[:, t:t + 1], axis=0))
                    # one-hot (edge-partition) and transposed one-hot
                    nc.vector.tensor_tensor(
                        out=o_tiles[:, tt, :],
                        in0=crel[:, t:t + 1].to_broadcast([128, 128]),
                        in1=iota_free[:], op=mybir.AluOpType.is_equal)
                    ct_ps = ps.tile([128, 128], F32, tag="ctps")
                    nc.tensor.matmul(out=ct_ps[:], lhsT=ones_row[:],
                                     rhs=crelT_w[0:1, tt * 128:(tt + 1) * 128],
                                     start=True, stop=True)
                    oT = wp.tile([128, 128], F32, tag="oT")
                    nc.vector.tensor_tensor(
                        out=oT[:], in0=iota_col[:].to_broadcast([128, 128]),
                        in1=ct_ps[:], op=mybir.AluOpType.is_equal)
                    nc.tensor.matmul(out=pw[:, tt * H:(tt + 1) * H], lhsT=oT[:],
                                     rhs=sc_sb[:, w, :], start=True, stop=True)
                    nc.tensor.matmul(out=pw[:, (MAXTW + tt) * H:(MAXTW + tt + 1) * H],
                                     lhsT=eaT_w[:, tt * 128:(tt + 1) * 128],
                                     rhs=a_sb[:], start=True, stop=True)
                # alpha = scg + s_r + se ; lrelu ; exp
                nc.vector.scalar_tensor_tensor(
                    out=al[:, :tw, :],
                    in0=pw[:, 0:tw * H].rearrange("p (t h) -> p t h", h=H),
                    scalar=0.0,
                    in1=hg[:, :tw, 64:72], op0=mybir.AluOpType.add,
                    op1=mybir.AluOpType.add)
                nc.vector.tensor_tensor(
                    out=al[:, :tw, :], in0=al[:, :tw, :],
                    in1=pw[:, MAXTW * H:(MAXTW + tw) * H].rearrange(
                        "p (t h) -> p t h", h=H),
                    op=mybir.AluOpType.add)
                nc.scalar.activation(out=al[:, :tw, :], in_=al[:, :tw, :],
                                     func=mybir.ActivationFunctionType.Lrelu, alpha=0.01)
                nc.scalar.activation(out=pay[:, :tw, 64:72], in_=al[:, :tw, :],
                                     func=mybir.ActivationFunctionType.Exp)
                # msg = ex * h
                nc.vector.tensor_tensor(
                    out=pay[:, :tw, 0:64].rearrange("p t (h d) -> p t h d", h=H),
                    in0=hg[:, :tw, 0:64].rearrange("p t (h d) -> p t h d", h=H),
                    in1=pay[:, :tw, 64:72].rearrange("p t (h x) -> p t h x", x=1).to_broadcast(
                        [128, tw, H, HD]),
                    op=mybir.AluOpType.mult)
                # scatter: window accumulation in PSUM
                W0 = 2 * MAXTW * H
                for tt in range(tw):
                    nc.tensor.matmul(out=pw[:, W0:W0 + 72], lhsT=o_tiles[:, tt, :],
                                     rhs=pay[:, tt, :], start=(tt == 0),
                                     stop=(tt == tw - 1))
                nc.vector.tensor_copy(out=tagg[:, w, :], in_=pw[:, W0:W0 + 72])

            # ---------------- P4: normalize + output -----------------------
            den = wp.tile([128, NWIN, H], F32, tag="den")
            nc.vector.tensor_scalar(out=den[:], in0=tagg[:, :, 64:72],
                                    scalar1=1e-16, scalar2=None,
                                    op0=mybir.AluOpType.add)
            nc.vector.reciprocal(out=den[:], in_=den[:])
            nc.vector.tensor_tensor(
                out=tagg[:, :, 0:64].rearrange("p w (h d) -> p w h d", h=H),
                in0=tagg[:, :, 0:64].rearrange("p w (h d) -> p w h d", h=H),
                in1=den[:].rearrange("p w (h x) -> p w h x", x=1).to_broadcast(
                    [128, NWIN, H, HD]),
                op=mybir.AluOpType.mult)
            nc.vector.tensor_tensor(
                out=tagg[:, :, 0:64], in0=tagg[:, :, 0:64],
                in1=bias_rep.rearrange("p (x d) -> p x d", x=1).to_broadcast([128, NWIN, D]),
                op=mybir.AluOpType.add)
            for w in range(NWIN):
                xt_ps = ps.tile([D, 128], F32, tag="pbig")
                nc.tensor.transpose(out=xt_ps[:], in_=tagg[:, w, 0:64],
                                    identity=ident[:])
                xt = wp.tile([D, 128], F32, tag="xt")
                nc.vector.tensor_copy(out=xt[:], in_=xt_ps[:])
                ow_ps = ps.tile([128, D], F32, tag="pmed")
                nc.tensor.matmul(out=ow_ps[:], lhsT=xt[:], rhs=woutT[:],
                                 start=True, stop=True)
                ow = wp.tile([128, D], F32, tag="ow")
                nc.vector.tensor_tensor(out=ow[:], in0=ow_ps[:], in1=bout_rep,
                                        op=mybir.AluOpType.add)
                nc.sync.dma_start(out=out_d[w * 128:(w + 1) * 128, :], in_=ow[:])

    _split_multi_waits(nc)
    return nc


# ---------------------------------------------------------------------------
_CACHE = {}
LAST_EXEC_NS = None


def kernel(feats, edge_index, edge_attr, W_fc, b_fc, W_edge, att, bias, W_out, b_out):
    feats_pad, cores, tiles_w, win_start, T = _host_prep(feats, edge_index, edge_attr)
    bd_c, bd_r, bd_e = _expand_att(att)

    key = ("v1", T, tuple(tiles_w.tolist()))
    if key not in _CACHE:
        _CACHE[key] = _build_nc(tiles_w, win_start, T)
    nc = _CACHE[key]

    shared = {
        "feats_pad": feats_pad,
        "W_fc": np.asarray(W_fc, np.float32),
        "b_fc_col": np.asarray(b_fc, np.float32).reshape(D, 1),
        "W_edge": np.asarray(W_edge, np.float32),
        "att_bd_c": bd_c, "att_bd_r": bd_r, "att_bd_e": bd_e,
        "bias_row": np.asarray(bias, np.float32).reshape(1, D),
        "W_out": np.asarray(W_out, np.float32),
        "b_out_row": np.asarray(b_out, np.float32).reshape(1, D),
    }
    in_maps = []
    for k in range(NCORES):
        m = dict(shared)
        m["feats_own"] = cores[k]["feats_own"]
        m["r_lay"] = cores[k]["r_lay"]
        m["crel_lay"] = cores[k]["crel_lay"]
        m["crelT_lay"] = cores[k]["crelT_lay"]
        m["eaT_lay"] = cores[k]["eaT_lay"]
        in_maps.append(m)

    trace = bool(int(os.environ.get("KERNEL_TRACE", "0")))
    res = run_bass_kernel_spmd(nc, in_maps, list(range(NCORES)), trace=trace)
    global LAST_EXEC_NS
    LAST_EXEC_NS = res.exec_time_ns
    out = np.concatenate(
        [res.results[k]["out_shard"][:NSH] for k in range(NCORES)], axis=0)
    return (out,
            np.asarray(edge_index),
            np.asarray(edge_attr, np.float32))


# revision 9
# speedup vs baseline: 1.2621x; 1.2621x over previous
"""Trainium2 Bass kernel for nn_MultiHeadMLP (GAT-style message passing).

Strategy (8 NeuronCores, SPMD):
  - Host shards edges by target-node range (12500 nodes per core) and sorts
    each shard by target; every per-edge array is a pure index-derived
    relayout of the inputs.
  - Each core builds the full node table [h | s_r] (redundantly), its own
    s_c table, then streams its edge shard: h[r] rows arrive via indirect
    DMA; the per-target one-hot matrices (built with is_equal against an
    iota) turn the segment softmax numerator/denominator sums into PE
    matmuls accumulated per 128-node window in PSUM.
  - Softmax max-subtraction is skipped: alpha magnitudes here are < 1, so
    exp() is numerically safe and the result is mathematically identical.
  - Output: normalize, add bias, multiply by W_out^T, add b_out; the host
    concatenates the 8 shards.
"""
import numpy as np
import os
import sys

if "/opt/trn_rl_repo" not in sys.path:
    sys.path.insert(0, "/opt/trn_rl_repo")

import concourse.bass as bass
import concourse.mybir as mybir
import concourse.tile as tile
from concourse.bass_utils import run_bass_kernel_spmd
from concourse.masks import make_identity

F32 = mybir.dt.float32
F32R = mybir.dt.float32r


def _r(ap):
    return ap.bitcast(F32R)
I32 = mybir.dt.int32

N = 100000
D = 64
H = 8
HD = 8
ED = 32
EHD = 4
NCORES = 8
NSH = N // NCORES            # 12500
NSH_PAD = 12544              # 98 * 128
NWIN = NSH_PAD // 128        # 98
N_PAD = 100352               # 784 * 128
PAD_CREL = 999.0


# ---------------------------------------------------------------------------
# walrus workaround: this build rejects instructions with >1 semaphore wait
def _split_multi_waits(nc):
    def _split_block(bb):
        insts = list(bb.instructions)
        out = []
        changed = False
        for inst in insts:
            si = inst.sync_info
            if si is not None:
                waits = list(si.on_wait)
                if len(waits) > 1:
                    for w in waits[:-1]:
                        out.append(mybir.InstNoOp(
                            name=nc.get_next_instruction_name(),
                            engine=inst.engine, ins=[], outs=[],
                            sync_info=mybir.SyncInfo(on_wait=[w], on_update=[]),
                        ))
                    inst.sync_info = mybir.SyncInfo(
                        on_wait=[waits[-1]], on_update=list(si.on_update))
                    changed = True
            out.append(inst)
        if changed:
            bb.instructions = out

    def _walk(blocks):
        for b in blocks:
            if hasattr(b, "instructions"):
                _split_block(b)
            if hasattr(b, "blocks"):
                _walk(b.blocks)

    for f in nc.m.functions:
        _walk(f.blocks)


# ---------------------------------------------------------------------------
def _host_prep(feats, edge_index, edge_attr):
    """Shard by target range, sort by target, build device layouts."""
    r = np.asarray(edge_index[:, 0]).astype(np.int64)
    c = np.asarray(edge_index[:, 1]).astype(np.int64)
    ea = np.asarray(edge_attr, np.float32)
    feats = np.asarray(feats, np.float32)
    feats_pad = np.zeros((N_PAD, D), np.float32)
    feats_pad[:N] = feats

    shards = []
    tiles_w_all = np.zeros((NCORES, NWIN), np.int64)
    for k in range(NCORES):
        sel = np.nonzero((c // NSH) == k)[0]
        c_k = c[sel] - k * NSH
        order = np.argsort(c_k, kind="stable")
        r_s = r[sel][order]
        c_s = c_k[order]
        ea_s = ea[sel][order]
        cnt = np.bincount(c_s // 128, minlength=NWIN)
        tiles_w_all[k] = (cnt + 127) // 128
        shards.append((r_s, c_s, ea_s, cnt))

    tiles_w = tiles_w_all.max(axis=0)          # common SPMD schedule
    win_start = np.zeros(NWIN + 1, np.int64)
    np.cumsum(tiles_w, out=win_start[1:])
    T = int(win_start[-1])
    Epad = T * 128

    cores = []
    for k in range(NCORES):
        r_s, c_s, ea_s, cnt = shards[k]
        r_pad = np.zeros(Epad, np.int32)
        crel_pad = np.full(Epad, PAD_CREL, np.float32)
        ea_pad = np.zeros((Epad, ED), np.float32)
        src = 0
        for wi in range(NWIN):
            n_e = int(cnt[wi])
            pos = int(win_start[wi]) * 128
            if n_e:
                r_pad[pos:pos + n_e] = r_s[src:src + n_e]
                crel_pad[pos:pos + n_e] = (c_s[src:src + n_e] - wi * 128)
                ea_pad[pos:pos + n_e] = ea_s[src:src + n_e]
            src += n_e
        cores.append(dict(
            r_lay=np.ascontiguousarray(r_pad.reshape(T, 128).T),      # [128,T] i32
            crel_lay=np.ascontiguousarray(crel_pad.reshape(T, 128).T),  # [128,T] f32
            crelT_lay=crel_pad.reshape(1, Epad).copy(),               # [1,128T] f32
            eaT_lay=np.ascontiguousarray(ea_pad.T),                   # [32,128T] f32
            feats_own=feats_pad[k * NSH:k * NSH + NSH_PAD].copy(),
        ))
    return feats_pad, cores, tiles_w, win_start, T


def _expand_att(att):
    """Block-diagonal relayout of att (values copied, no arithmetic)."""
    att = np.asarray(att, np.float32)[:, :, 0]            # [H, 2HD+EHD]
    bd_c = np.zeros((D, H), np.float32)
    bd_r = np.zeros((D, H), np.float32)
    bd_e = np.zeros((ED, H), np.float32)
    for h in range(H):
        bd_c[h * HD:(h + 1) * HD, h] = att[h, :HD]
        bd_r[h * HD:(h + 1) * HD, h] = att[h, HD:2 * HD]
        bd_e[h * EHD:(h + 1) * EHD, h] = att[h, 2 * HD:]
    return bd_c, bd_r, bd_e


# ---------------------------------------------------------------------------
def _build_nc(tiles_w, win_start, T):
    nc = bass.Bass()
    Epad = T * 128

    feats_d = nc.dram_tensor("feats_pad", [N_PAD, D], F32, kind="ExternalInput")
    fown_d = nc.dram_tensor("feats_own", [NSH_PAD, D], F32, kind="ExternalInput")
    wfc_d = nc.dram_tensor("W_fc", [D, D], F32, kind="ExternalInput")
    bfc_d = nc.dram_tensor("b_fc_col", [D, 1], F32, kind="ExternalInput")
    wedge_d = nc.dram_tensor("W_edge", [ED, ED], F32, kind="ExternalInput")
    bd_c_d = nc.dram_tensor("att_bd_c", [D, H], F32, kind="ExternalInput")
    bd_r_d = nc.dram_tensor("att_bd_r", [D, H], F32, kind="ExternalInput")
    bd_e_d = nc.dram_tensor("att_bd_e", [ED, H], F32, kind="ExternalInput")
    bias_d = nc.dram_tensor("bias_row", [1, D], F32, kind="ExternalInput")
    wout_d = nc.dram_tensor("W_out", [D, D], F32, kind="ExternalInput")
    bout_d = nc.dram_tensor("b_out_row", [1, D], F32, kind="ExternalInput")
    rlay_d = nc.dram_tensor("r_lay", [128, T], I32, kind="ExternalInput")
    crel_d = nc.dram_tensor("crel_lay", [128, T], F32, kind="ExternalInput")
    crelT_d = nc.dram_tensor("crelT_lay", [1, Epad], F32, kind="ExternalInput")
    eaT_d = nc.dram_tensor("eaT_lay", [ED, Epad], F32, kind="ExternalInput")

    ttab_d = nc.dram_tensor("T_tab", [N_PAD, 72], F32)     # [h | s_r]
    out_d = nc.dram_tensor("out_shard", [NSH_PAD, D], F32, kind="ExternalOutput")

    with tile.TileContext(nc) as tc:
        with tc.tile_pool(name="const", bufs=1) as cp, \
             tc.tile_pool(name="work", bufs=3) as wp, \
             tc.tile_pool(name="hgp", bufs=2) as hgp, \
             tc.tile_pool(name="ps", bufs=2, space="PSUM") as ps:

            # ---------------- P0: constants & small weights ----------------
            ident = cp.tile([128, 128], F32)
            make_identity(nc, ident[:])
            iota_i = cp.tile([128, 128], I32, tag="ioi")
            nc.gpsimd.iota(iota_i[:], pattern=[[1, 128]], channel_multiplier=0)
            iota_free = cp.tile([128, 128], F32)           # each row = 0..127
            nc.vector.tensor_copy(out=iota_free[:], in_=iota_i[:])
            iota_ci = cp.tile([128, 1], I32, tag="ioc")
            nc.gpsimd.iota(iota_ci[:], pattern=[[0, 1]], channel_multiplier=1)
            iota_col = cp.tile([128, 1], F32)              # partition index
            nc.vector.tensor_copy(out=iota_col[:], in_=iota_ci[:])
            ones_row = cp.tile([1, 128], F32)
            nc.vector.memset(ones_row[:], 1.0)

            wfc = cp.tile([D, D], F32)
            nc.sync.dma_start(out=wfc[:], in_=wfc_d[:])
            wedge = cp.tile([ED, ED], F32)
            nc.sync.dma_start(out=wedge[:], in_=wedge_d[:])
            bdc = cp.tile([D, H], F32)
            nc.sync.dma_start(out=bdc[:], in_=bd_c_d[:])
            bdr = cp.tile([D, H], F32)
            nc.sync.dma_start(out=bdr[:], in_=bd_r_d[:])
            bde = cp.tile([ED, H], F32)
            nc.sync.dma_start(out=bde[:], in_=bd_e_d[:])
            bfc_col = cp.tile([D, 1], F32)
            nc.sync.dma_start(out=bfc_col[:], in_=bfc_d[:])
            wout = cp.tile([D, D], F32)
            nc.sync.dma_start(out=wout[:], in_=wout_d[:])

            # R1 = [Wfc^T | B_r]  (rhs for the node-table matmul)
            r1_ps = ps.tile([D, 72], F32, tag="pmed")
            nc.tensor.transpose(out=r1_ps[:, 0:D], in_=wfc[:], identity=ident[0:D, 0:D])
            nc.tensor.matmul(out=r1_ps[:, D:72], lhsT=wfc[:], rhs=bdr[:],
                             start=True, stop=True)
            r1 = cp.tile([D, 72], F32R)
            nc.vector.tensor_copy(out=r1[:], in_=r1_ps[:])
            # B_c, A_edge, W_out^T
            r2_ps = ps.tile([D, H], F32, tag="pmed")
            nc.tensor.matmul(out=r2_ps[:], lhsT=wfc[:], rhs=bdc[:], start=True, stop=True)
            r2 = cp.tile([D, H], F32R)
            nc.vector.tensor_copy(out=r2[:], in_=r2_ps[:])
            ae_ps = ps.tile([ED, H], F32, tag="pmed")
            nc.tensor.matmul(out=ae_ps[:], lhsT=wedge[:], rhs=bde[:], start=True, stop=True)
            a_sb = cp.tile([ED, H], F32)
            nc.vector.tensor_copy(out=a_sb[:], in_=ae_ps[:])
            woutT_ps = ps.tile([D, D], F32, tag="pmed")
            nc.tensor.transpose(out=woutT_ps[:], in_=wout[:], identity=ident[0:D, 0:D])
            woutT = cp.tile([D, D], F32R)
            nc.vector.tensor_copy(out=woutT[:], in_=woutT_ps[:])

            # bias rows: [b_fc | b_r_const] and b_c_const
            brow_ps = ps.tile([1, 72 + H], F32, tag="pmed")
            nc.tensor.matmul(out=brow_ps[:, D:72], lhsT=bfc_col[:], rhs=bdr[:],
                             start=True, stop=True)
            nc.tensor.matmul(out=brow_ps[:, 72:72 + H], lhsT=bfc_col[:], rhs=bdc[:],
                             start=True, stop=True)
            brow = cp.tile([1, 72 + H], F32)    # [:64]=b_fc, [64:72]=b_r, [72:80]=b_c
            nc.sync.dma_start(out=brow[0:1, 0:D], in_=bfc_d[:].rearrange("a b -> b a"))
            nc.vector.tensor_copy(out=brow[:, D:72 + H], in_=brow_ps[:, D:72 + H])
            # replicate to 128 partitions
            brep_ps = ps.tile([128, 72 + H + 2 * D], F32, tag="pmed")
            nc.tensor.matmul(out=brep_ps[:, :72 + H], lhsT=ones_row[:], rhs=brow[:],
                             start=True, stop=True)
            bias_row = cp.tile([1, D], F32)
            nc.sync.dma_start(out=bias_row[:], in_=bias_d[:])
            bout_row = cp.tile([1, D], F32)
            nc.sync.dma_start(out=bout_row[:], in_=bout_d[:])
            nc.tensor.matmul(out=brep_ps[:, 72 + H:72 + H + D], lhsT=ones_row[:],
                             rhs=bias_row[:], start=True, stop=True)
            nc.tensor.matmul(out=brep_ps[:, 72 + H + D:], lhsT=ones_row[:],
                             rhs=bout_row[:], start=True, stop=True)
            brep = cp.tile([128, 72 + H + 2 * D], F32)
            nc.vector.tensor_copy(out=brep[:], in_=brep_ps[:])
            # views
            bfc72_rep = brep[:, 0:72]       # node-table bias [b_fc | b_r]
            bc_rep = brep[:, 72:72 + H]
            bias_rep = brep[:, 72 + H:72 + H + D]
            bout_rep = brep[:, 72 + H + D:72 + H + 2 * D]

            # ---------------- P1: global node table T = [h | s_r] ----------
            NG = N_PAD // 512
            for g in range(NG):
                f4 = wp.tile([128, 4, D], F32, tag="f4")
                nc.sync.dma_start(
                    out=f4[:],
                    in_=feats_d[:].rearrange("(g j p) d -> g p j d", j=4, p=128)[g])
                ft_ps = ps.tile([D, 512], F32, tag="pbig")
                for j in range(4):
                    nc.tensor.transpose(out=ft_ps[:, j * 128:(j + 1) * 128],
                                        in_=f4[:, j, :], identity=ident[:])
                ft = wp.tile([D, 512], F32R, tag="ft")
                nc.vector.tensor_copy(out=ft[:], in_=ft_ps[:])
                o_ps = ps.tile([128, 4, 72], F32, tag="pmed")
                for j in range(4):
                    nc.tensor.matmul(out=o_ps[:, j, :], lhsT=ft[:, j * 128:(j + 1) * 128],
                                     rhs=r1[:], start=True, stop=True)
                o_sb = wp.tile([128, 4, 72], F32, tag="osb")
                nc.vector.tensor_tensor(
                    out=o_sb[:], in0=o_ps[:],
                    in1=bfc72_rep.rearrange("p (x d) -> p x d", x=1).to_broadcast([128, 4, 72]),
                    op=mybir.AluOpType.add)
                nc.sync.dma_start(
                    out=ttab_d[:].rearrange("(g j p) d -> g p j d", j=4, p=128)[g],
                    in_=o_sb[:])

            # ---------------- P2: own s_c table (SBUF-resident) ------------
            sc_sb = cp.tile([128, NWIN, H], F32R)
            NG2 = NSH_PAD // 256
            for g in range(NG2):
                f4 = wp.tile([128, 2, D], F32, tag="f4b")
                nc.sync.dma_start(
                    out=f4[:],
                    in_=fown_d[:].rearrange("(g j p) d -> g p j d", j=2, p=128)[g])
                ft_ps = ps.tile([D, 256], F32, tag="pbig")
                for j in range(2):
                    nc.tensor.transpose(out=ft_ps[:, j * 128:(j + 1) * 128],
                                        in_=f4[:, j, :], identity=ident[:])
                ft = wp.tile([D, 256], F32R, tag="ft2")
                nc.vector.tensor_copy(out=ft[:], in_=ft_ps[:])
                s_ps = ps.tile([128, 2, H], F32, tag="pmed")
                for j in range(2):
                    nc.tensor.matmul(out=s_ps[:, j, :], lhsT=ft[:, j * 128:(j + 1) * 128],
                                     rhs=r2[:], start=True, stop=True)
                nc.vector.tensor_tensor(
                    out=sc_sb[:, 2 * g:2 * g + 2, :], in0=s_ps[:],
                    in1=bc_rep.rearrange("p (x d) -> p x d", x=1).to_broadcast([128, 2, H]),
                    op=mybir.AluOpType.add)

            # ---------------- P3: edge phase -------------------------------
            ridx = cp.tile([128, T], I32)
            nc.sync.dma_start(out=ridx[:], in_=rlay_d[:])
            crel = cp.tile([128, T], F32)
            nc.sync.dma_start(out=crel[:], in_=crel_d[:])
            tagg = cp.tile([128, NWIN, 72], F32)

            MAXTW = int(tiles_w.max())
            for w in range(NWIN):
                tw = int(tiles_w[w])
                t0 = int(win_start[w])
                if tw == 0:
                    continue
                crelT_w = wp.tile([1, MAXTW * 128], F32, tag="crelTw")
                nc.sync.dma_start(out=crelT_w[0:1, :tw * 128],
                                  in_=crelT_d[0:1, t0 * 128:(t0 + tw) * 128])
                eaT_w = wp.tile([ED, MAXTW * 128], F32, tag="eaTw")
                nc.sync.dma_start(out=eaT_w[:, :tw * 128],
                                  in_=eaT_d[:, t0 * 128:(t0 + tw) * 128])
                hg = hgp.tile([128, MAXTW, 72], F32, tag="hg")
                pw = ps.tile([128, 2 * MAXTW * H + 72], F32, tag="pswin")
                pay = hgp.tile([128, MAXTW, 72], F32R, tag="pay")
                al = wp.tile([128, MAXTW, H], F32, tag="al")
                o_tiles = wp.tile([128, MAXTW, 128], F32R, tag="otl")

                for tt in range(tw):
                    t = t0 + tt
                    nc.gpsimd.indirect_dma_start(
                        out=hg[:, tt, :], out_offset=None, in_=ttab_d[:],
                        in_offset=bass.IndirectOffsetOnAxis(ap=ridx[:, t:t + 1], axis=0))
                    # one-hot (edge-partition) and transposed one-hot
                    nc.vector.tensor_tensor(
                        out=o_tiles[:, tt, :],
                        in0=crel[:, t:t + 1].to_broadcast([128, 128]),
                        in1=iota_free[:], op=mybir.AluOpType.is_equal)
                    ct_ps = ps.tile([128, 128], F32, tag="ctps")
                    nc.tensor.matmul(out=ct_ps[:], lhsT=ones_row[:],
                                     rhs=crelT_w[0:1, tt * 128:(tt + 1) * 128],
                                     start=True, stop=True)
                    oT = wp.tile([128, 128], F32R, tag="oT")
                    nc.vector.tensor_tensor(
                        out=oT[:], in0=iota_col[:].to_broadcast([128, 128]),
                        in1=ct_ps[:], op=mybir.AluOpType.is_equal)
                    nc.tensor.matmul(out=pw[:, tt * H:(tt + 1) * H], lhsT=oT[:],
                                     rhs=sc_sb[:, w, :], start=True, stop=True)
                    nc.tensor.matmul(out=pw[:, (MAXTW + tt) * H:(MAXTW + tt + 1) * H],
                                     lhsT=eaT_w[:, tt * 128:(tt + 1) * 128],
                                     rhs=a_sb[:], start=True, stop=True)
                # alpha = scg + s_r + se ; lrelu ; exp
                nc.vector.scalar_tensor_tensor(
                    out=al[:, :tw, :],
                    in0=pw[:, 0:tw * H].rearrange("p (t h) -> p t h", h=H),
                    scalar=0.0,
                    in1=hg[:, :tw, 64:72], op0=mybir.AluOpType.add,
                    op1=mybir.AluOpType.add)
                nc.vector.tensor_tensor(
                    out=al[:, :tw, :], in0=al[:, :tw, :],
                    in1=pw[:, MAXTW * H:(MAXTW + tw) * H].rearrange(
                        "p (t h) -> p t h", h=H),
                    op=mybir.AluOpType.add)
                nc.scalar.activation(out=al[:, :tw, :], in_=al[:, :tw, :],
                                     func=mybir.ActivationFunctionType.Lrelu, alpha=0.01)
                nc.scalar.activation(out=pay[:, :tw, 64:72], in_=al[:, :tw, :],
                                     func=mybir.ActivationFunctionType.Exp)
                # msg = ex * h
                nc.vector.tensor_tensor(
                    out=pay[:, :tw, 0:64].rearrange("p t (h d) -> p t h d", h=H),
                    in0=hg[:, :tw, 0:64].rearrange("p t (h d) -> p t h d", h=H),
                    in1=pay[:, :tw, 64:72].bitcast(F32).rearrange(
                        "p t (h x) -> p t h x", x=1).to_broadcast([128, tw, H, HD]),
                    op=mybir.AluOpType.mult)
                # scatter: window accumulation in PSUM
                W0 = 2 * MAXTW * H
                for tt in range(tw):
                    nc.tensor.matmul(out=pw[:, W0:W0 + 72], lhsT=o_tiles[:, tt, :],
                                     rhs=pay[:, tt, :], start=(tt == 0),
                                     stop=(tt == tw - 1))
                nc.vector.tensor_copy(out=tagg[:, w, :], in_=pw[:, W0:W0 + 72])

            # ---------------- P4: normalize + output -----------------------
            den = wp.tile([128, NWIN, H], F32, tag="den")
            nc.vector.tensor_scalar(out=den[:], in0=tagg[:, :, 64:72],
                                    scalar1=1e-16, scalar2=None,
                                    op0=mybir.AluOpType.add)
            nc.vector.reciprocal(out=den[:], in_=den[:])
            nc.vector.tensor_tensor(
                out=tagg[:, :, 0:64].rearrange("p w (h d) -> p w h d", h=H),
                in0=tagg[:, :, 0:64].rearrange("p w (h d) -> p w h d", h=H),
                in1=den[:].rearrange("p w (h x) -> p w h x", x=1).to_broadcast(
                    [128, NWIN, H, HD]),
                op=mybir.AluOpType.mult)
            nc.vector.tensor_tensor(
                out=tagg[:, :, 0:64], in0=tagg[:, :, 0:64],
                in1=bias_rep.rearrange("p (x d) -> p x d", x=1).to_broadcast([128, NWIN, D]),
                op=mybir.AluOpType.add)
            for w in range(NWIN):
                xt_ps = ps.tile([D, 128], F32, tag="pbig")
                nc.tensor.transpose(out=xt_ps[:], in_=tagg[:, w, 0:64],
                                    identity=ident[:])
                xt = wp.tile([D, 128], F32R, tag="xt")
                nc.vector.tensor_copy(out=xt[:], in_=xt_ps[:])
                ow_ps = ps.tile([128, D], F32, tag="pmed")
                nc.tensor.matmul(out=ow_ps[:], lhsT=xt[:], rhs=woutT[:],
                                 start=True, stop=True)
                ow = wp.tile([128, D], F32, tag="ow")
                nc.vector.tensor_tensor(out=ow[:], in0=ow_ps[:], in1=bout_rep,
                                        op=mybir.AluOpType.add)
                nc.sync.dma_start(out=out_d[w * 128:(w + 1) * 128, :], in_=ow[:])

    _split_multi_waits(nc)
    return nc


# ---------------------------------------------------------------------------
_CACHE = {}
LAST_EXEC_NS = None


def kernel(feats, edge_index, edge_attr, W_fc, b_fc, W_edge, att, bias, W_out, b_out):
    feats_pad, cores, tiles_w, win_start, T = _host_prep(feats, edge_index, edge_attr)
    bd_c, bd_r, bd_e = _expand_att(att)

    key = ("v3", T, tuple(tiles_w.tolist()))
    if key not in _CACHE:
        _CACHE[key] = _build_nc(tiles_w, win_start, T)
    nc = _CACHE[key]

    shared = {
        "feats_pad": feats_pad,
        "W_fc": np.asarray(W_fc, np.float32),
        "b_fc_col": np.asarray(b_fc, np.float32).reshape(D, 1),
        "W_edge": np.asarray(W_edge, np.float32),
        "att_bd_c": bd_c, "att_bd_r": bd_r, "att_bd_e": bd_e,
        "bias_row": np.asarray(bias, np.float32).reshape(1, D),
        "W_out": np.asarray(W_out, np.float32),
        "b_out_row": np.asarray(b_out, np.float32).reshape(1, D),
    }
    in_maps = []
    for k in range(NCORES):
        m = dict(shared)
        m["feats_own"] = cores[k]["feats_own"]
        m["r_lay"] = cores[k]["r_lay"]
        m["crel_lay"] = cores[k]["crel_lay"]
        m["crelT_lay"] = cores[k]["crelT_lay"]
        m["eaT_lay"] = cores[k]["eaT_lay"]
        in_maps.append(m)

    trace = bool(int(os.environ.get("KERNEL_TRACE", "0")))
    res = run_bass_kernel_spmd(nc, in_maps, list(range(NCORES)), trace=trace)
    global LAST_EXEC_NS
    LAST_EXEC_NS = res.exec_time_ns
    out = np.concatenate(
        [res.results[k]["out_shard"][:NSH] for k in range(NCORES)], axis=0)
    return (out,
            np.asarray(edge_index),
            np.asarray(edge_attr, np.float32))


# revision 10
# speedup vs baseline: 1.4491x; 1.1482x over previous
"""Trainium2 Bass kernel for nn_MultiHeadMLP (GAT-style message passing).

Strategy (8 NeuronCores, SPMD):
  - Host shards edges by target-node range (12500 nodes per core) and sorts
    each shard by target; every per-edge array is a pure index-derived
    relayout of the inputs.
  - Each core builds the full node table [h | s_r] (redundantly), its own
    s_c table, then streams its edge shard: h[r] rows arrive via indirect
    DMA; the per-target one-hot matrices (built with is_equal against an
    iota) turn the segment softmax numerator/denominator sums into PE
    matmuls accumulated per 128-node window in PSUM.
  - Softmax max-subtraction is skipped: alpha magnitudes here are < 1, so
    exp() is numerically safe and the result is mathematically identical.
  - Output: normalize, add bias, multiply by W_out^T, add b_out; the host
    concatenates the 8 shards.
"""
import numpy as np
import os
import sys

if "/opt/trn_rl_repo" not in sys.path:
    sys.path.insert(0, "/opt/trn_rl_repo")

import concourse.bass as bass
import concourse.mybir as mybir
import concourse.tile as tile
from concourse.bass_utils import run_bass_kernel_spmd
from concourse.masks import make_identity

F32 = mybir.dt.float32
F32R = mybir.dt.float32r
BF16 = mybir.dt.bfloat16


def _r(ap):
    return ap.bitcast(F32R)
I32 = mybir.dt.int32

N = 100000
D = 64
H = 8
HD = 8
ED = 32
EHD = 4
NCORES = 8
NSH = N // NCORES            # 12500
NSH_PAD = 12544              # 98 * 128
NWIN = NSH_PAD // 128        # 98
N_PAD = 100352               # 784 * 128
PAD_CREL = 999.0


# ---------------------------------------------------------------------------
# walrus workaround: this build rejects instructions with >1 semaphore wait
def _split_multi_waits(nc):
    def _split_block(bb):
        insts = list(bb.instructions)
        out = []
        changed = False
        for inst in insts:
            si = inst.sync_info
            if si is not None:
                waits = list(si.on_wait)
                if len(waits) > 1:
                    for w in waits[:-1]:
                        out.append(mybir.InstNoOp(
                            name=nc.get_next_instruction_name(),
                            engine=inst.engine, ins=[], outs=[],
                            sync_info=mybir.SyncInfo(on_wait=[w], on_update=[]),
                        ))
                    inst.sync_info = mybir.SyncInfo(
                        on_wait=[waits[-1]], on_update=list(si.on_update))
                    changed = True
            out.append(inst)
        if changed:
            bb.instructions = out

    def _walk(blocks):
        for b in blocks:
            if hasattr(b, "instructions"):
                _split_block(b)
            if hasattr(b, "blocks"):
                _walk(b.blocks)

    for f in nc.m.functions:
        _walk(f.blocks)


# ---------------------------------------------------------------------------
def _host_prep(feats, edge_index, edge_attr):
    """Shard by target range, sort by target, build device layouts."""
    r = np.asarray(edge_index[:, 0]).astype(np.int64)
    c = np.asarray(edge_index[:, 1]).astype(np.int64)
    ea = np.asarray(edge_attr, np.float32)
    feats = np.asarray(feats, np.float32)
    feats_pad = np.zeros((N_PAD, D), np.float32)
    feats_pad[:N] = feats

    shards = []
    tiles_w_all = np.zeros((NCORES, NWIN), np.int64)
    for k in range(NCORES):
        sel = np.nonzero((c // NSH) == k)[0]
        c_k = c[sel] - k * NSH
        order = np.argsort(c_k, kind="stable")
        r_s = r[sel][order]
        c_s = c_k[order]
        ea_s = ea[sel][order]
        cnt = np.bincount(c_s // 128, minlength=NWIN)
        tiles_w_all[k] = (cnt + 127) // 128
        shards.append((r_s, c_s, ea_s, cnt))

    tiles_w = tiles_w_all.max(axis=0)          # common SPMD schedule
    win_start = np.zeros(NWIN + 1, np.int64)
    np.cumsum(tiles_w, out=win_start[1:])
    T = int(win_start[-1])
    Epad = T * 128

    cores = []
    for k in range(NCORES):
        r_s, c_s, ea_s, cnt = shards[k]
        r_pad = np.zeros(Epad, np.int32)
        crel_pad = np.full(Epad, PAD_CREL, np.float32)
        ea_pad = np.zeros((Epad, ED), np.float32)
        src = 0
        for wi in range(NWIN):
            n_e = int(cnt[wi])
            pos = int(win_start[wi]) * 128
            if n_e:
                r_pad[pos:pos + n_e] = r_s[src:src + n_e]
                crel_pad[pos:pos + n_e] = (c_s[src:src + n_e] - wi * 128)
                ea_pad[pos:pos + n_e] = ea_s[src:src + n_e]
            src += n_e
        cores.append(dict(
            r_lay=np.ascontiguousarray(r_pad.reshape(T, 128).T),      # [128,T] i32
            crel_lay=np.ascontiguousarray(crel_pad.reshape(T, 128).T),  # [128,T] f32
            crelT_lay=crel_pad.reshape(1, Epad).copy(),               # [1,128T] f32
            eaT_lay=np.ascontiguousarray(ea_pad.T),                   # [32,128T] f32
            feats_own=feats_pad[k * NSH:k * NSH + NSH_PAD].copy(),
        ))
    return feats_pad, cores, tiles_w, win_start, T


def _expand_att(att):
    """Block-diagonal relayout of att (values copied, no arithmetic)."""
    att = np.asarray(att, np.float32)[:, :, 0]            # [H, 2HD+EHD]
    bd_c = np.zeros((D, H), np.float32)
    bd_r = np.zeros((D, H), np.float32)
    bd_e = np.zeros((ED, H), np.float32)
    for h in range(H):
        bd_c[h * HD:(h + 1) * HD, h] = att[h, :HD]
        bd_r[h * HD:(h + 1) * HD, h] = att[h, HD:2 * HD]
        bd_e[h * EHD:(h + 1) * EHD, h] = att[h, 2 * HD:]
    return bd_c, bd_r, bd_e


# ---------------------------------------------------------------------------
def _build_nc(tiles_w, win_start, T):
    nc = bass.Bass()
    Epad = T * 128

    feats_d = nc.dram_tensor("feats_pad", [N_PAD, D], F32, kind="ExternalInput")
    fown_d = nc.dram_tensor("feats_own", [NSH_PAD, D], F32, kind="ExternalInput")
    wfc_d = nc.dram_tensor("W_fc", [D, D], F32, kind="ExternalInput")
    bfc_d = nc.dram_tensor("b_fc_col", [D, 1], F32, kind="ExternalInput")
    wedge_d = nc.dram_tensor("W_edge", [ED, ED], F32, kind="ExternalInput")
    bd_c_d = nc.dram_tensor("att_bd_c", [D, H], F32, kind="ExternalInput")
    bd_r_d = nc.dram_tensor("att_bd_r", [D, H], F32, kind="ExternalInput")
    bd_e_d = nc.dram_tensor("att_bd_e", [ED, H], F32, kind="ExternalInput")
    bias_d = nc.dram_tensor("bias_row", [1, D], F32, kind="ExternalInput")
    wout_d = nc.dram_tensor("W_out", [D, D], F32, kind="ExternalInput")
    bout_d = nc.dram_tensor("b_out_row", [1, D], F32, kind="ExternalInput")
    rlay_d = nc.dram_tensor("r_lay", [128, T], I32, kind="ExternalInput")
    crel_d = nc.dram_tensor("crel_lay", [128, T], F32, kind="ExternalInput")
    crelT_d = nc.dram_tensor("crelT_lay", [1, Epad], F32, kind="ExternalInput")
    eaT_d = nc.dram_tensor("eaT_lay", [ED, Epad], F32, kind="ExternalInput")

    ttab_d = nc.dram_tensor("T_tab", [N_PAD, 72], BF16)     # [h | s_r]
    out_d = nc.dram_tensor("out_shard", [NSH_PAD, D], F32, kind="ExternalOutput")

    with tile.TileContext(nc) as tc:
        with tc.tile_pool(name="const", bufs=1) as cp, \
             tc.tile_pool(name="work", bufs=3) as wp, \
             tc.tile_pool(name="hgp", bufs=2) as hgp, \
             tc.tile_pool(name="ps", bufs=2, space="PSUM") as ps:

            # ---------------- P0: constants & small weights ----------------
            ident = cp.tile([128, 128], F32)
            make_identity(nc, ident[:])
            iota_i = cp.tile([128, 128], I32, tag="ioi")
            nc.gpsimd.iota(iota_i[:], pattern=[[1, 128]], channel_multiplier=0)
            iota_free = cp.tile([128, 128], F32)           # each row = 0..127
            nc.vector.tensor_copy(out=iota_free[:], in_=iota_i[:])
            iota_ci = cp.tile([128, 1], I32, tag="ioc")
            nc.gpsimd.iota(iota_ci[:], pattern=[[0, 1]], channel_multiplier=1)
            iota_col = cp.tile([128, 1], F32)              # partition index
            nc.vector.tensor_copy(out=iota_col[:], in_=iota_ci[:])
            ones_row = cp.tile([1, 128], F32)
            nc.vector.memset(ones_row[:], 1.0)

            wfc = cp.tile([D, D], F32)
            nc.sync.dma_start(out=wfc[:], in_=wfc_d[:])
            wedge = cp.tile([ED, ED], F32)
            nc.sync.dma_start(out=wedge[:], in_=wedge_d[:])
            bdc = cp.tile([D, H], F32)
            nc.sync.dma_start(out=bdc[:], in_=bd_c_d[:])
            bdr = cp.tile([D, H], F32)
            nc.sync.dma_start(out=bdr[:], in_=bd_r_d[:])
            bde = cp.tile([ED, H], F32)
            nc.sync.dma_start(out=bde[:], in_=bd_e_d[:])
            bfc_col = cp.tile([D, 1], F32)
            nc.sync.dma_start(out=bfc_col[:], in_=bfc_d[:])
            wout = cp.tile([D, D], F32)
            nc.sync.dma_start(out=wout[:], in_=wout_d[:])

            # R1 = [Wfc^T | B_r]  (rhs for the node-table matmul)
            r1_ps = ps.tile([D, 72], F32, tag="pmed")
            nc.tensor.transpose(out=r1_ps[:, 0:D], in_=wfc[:], identity=ident[0:D, 0:D])
            nc.tensor.matmul(out=r1_ps[:, D:72], lhsT=wfc[:], rhs=bdr[:],
                             start=True, stop=True)
            r1 = cp.tile([D, 72], F32R)
            nc.vector.tensor_copy(out=r1[:], in_=r1_ps[:])
            # B_c, A_edge, W_out^T
            r2_ps = ps.tile([D, H], F32, tag="pmed")
            nc.tensor.matmul(out=r2_ps[:], lhsT=wfc[:], rhs=bdc[:], start=True, stop=True)
            r2 = cp.tile([D, H], F32R)
            nc.vector.tensor_copy(out=r2[:], in_=r2_ps[:])
            ae_ps = ps.tile([ED, H], F32, tag="pmed")
            nc.tensor.matmul(out=ae_ps[:], lhsT=wedge[:], rhs=bde[:], start=True, stop=True)
            a_sb = cp.tile([ED, H], F32)
            nc.vector.tensor_copy(out=a_sb[:], in_=ae_ps[:])
            woutT_ps = ps.tile([D, D], F32, tag="pmed")
            nc.tensor.transpose(out=woutT_ps[:], in_=wout[:], identity=ident[0:D, 0:D])
            woutT = cp.tile([D, D], F32R)
            nc.vector.tensor_copy(out=woutT[:], in_=woutT_ps[:])

            # bias rows: [b_fc | b_r_const] and b_c_const
            brow_ps = ps.tile([1, 72 + H], F32, tag="pmed")
            nc.tensor.matmul(out=brow_ps[:, D:72], lhsT=bfc_col[:], rhs=bdr[:],
                             start=True, stop=True)
            nc.tensor.matmul(out=brow_ps[:, 72:72 + H], lhsT=bfc_col[:], rhs=bdc[:],
                             start=True, stop=True)
            brow = cp.tile([1, 72 + H], F32)    # [:64]=b_fc, [64:72]=b_r, [72:80]=b_c
            nc.sync.dma_start(out=brow[0:1, 0:D], in_=bfc_d[:].rearrange("a b -> b a"))
            nc.vector.tensor_copy(out=brow[:, D:72 + H], in_=brow_ps[:, D:72 + H])
            # replicate to 128 partitions
            brep_ps = ps.tile([128, 72 + H + 2 * D], F32, tag="pmed")
            nc.tensor.matmul(out=brep_ps[:, :72 + H], lhsT=ones_row[:], rhs=brow[:],
                             start=True, stop=True)
            bias_row = cp.tile([1, D], F32)
            nc.sync.dma_start(out=bias_row[:], in_=bias_d[:])
            bout_row = cp.tile([1, D], F32)
            nc.sync.dma_start(out=bout_row[:], in_=bout_d[:])
            nc.tensor.matmul(out=brep_ps[:, 72 + H:72 + H + D], lhsT=ones_row[:],
                             rhs=bias_row[:], start=True, stop=True)
            nc.tensor.matmul(out=brep_ps[:, 72 + H + D:], lhsT=ones_row[:],
                             rhs=bout_row[:], start=True, stop=True)
            brep = cp.tile([128, 72 + H + 2 * D], F32)
            nc.vector.tensor_copy(out=brep[:], in_=brep_ps[:])
            # views
            bfc72_rep = brep[:, 0:72]       # node-table bias [b_fc | b_r]
            bc_rep = brep[:, 72:72 + H]
            bias_rep = brep[:, 72 + H:72 + H + D]
            bout_rep = brep[:, 72 + H + D:72 + H + 2 * D]

            # ---------------- P1: global node table T = [h | s_r] ----------
            NG = N_PAD // 512
            for g in range(NG):
                f4 = wp.tile([128, 4, D], F32, tag="f4")
                nc.sync.dma_start(
                    out=f4[:],
                    in_=feats_d[:].rearrange("(g j p) d -> g p j d", j=4, p=128)[g])
                ft_ps = ps.tile([D, 512], F32, tag="pbig")
                for j in range(4):
                    nc.tensor.transpose(out=ft_ps[:, j * 128:(j + 1) * 128],
                                        in_=f4[:, j, :], identity=ident[:])
                ft = wp.tile([D, 512], F32R, tag="ft")
                nc.vector.tensor_copy(out=ft[:], in_=ft_ps[:])
                o_ps = ps.tile([128, 4, 72], F32, tag="pmed")
                for j in range(4):
                    nc.tensor.matmul(out=o_ps[:, j, :], lhsT=ft[:, j * 128:(j + 1) * 128],
                                     rhs=r1[:], start=True, stop=True)
                o_sb = wp.tile([128, 4, 72], BF16, tag="osb")
                nc.vector.tensor_tensor(
                    out=o_sb[:], in0=o_ps[:],
                    in1=bfc72_rep.rearrange("p (x d) -> p x d", x=1).to_broadcast([128, 4, 72]),
                    op=mybir.AluOpType.add)
                nc.sync.dma_start(
                    out=ttab_d[:].rearrange("(g j p) d -> g p j d", j=4, p=128)[g],
                    in_=o_sb[:])

            # ---------------- P2: own s_c table (SBUF-resident) ------------
            sc_sb = cp.tile([128, NWIN, H], BF16)
            NG2 = NSH_PAD // 256
            for g in range(NG2):
                f4 = wp.tile([128, 2, D], F32, tag="f4b")
                nc.sync.dma_start(
                    out=f4[:],
                    in_=fown_d[:].rearrange("(g j p) d -> g p j d", j=2, p=128)[g])
                ft_ps = ps.tile([D, 256], F32, tag="pbig")
                for j in range(2):
                    nc.tensor.transpose(out=ft_ps[:, j * 128:(j + 1) * 128],
                                        in_=f4[:, j, :], identity=ident[:])
                ft = wp.tile([D, 256], F32R, tag="ft2")
                nc.vector.tensor_copy(out=ft[:], in_=ft_ps[:])
                s_ps = ps.tile([128, 2, H], F32, tag="pmed")
                for j in range(2):
                    nc.tensor.matmul(out=s_ps[:, j, :], lhsT=ft[:, j * 128:(j + 1) * 128],
                                     rhs=r2[:], start=True, stop=True)
                nc.vector.tensor_tensor(
                    out=sc_sb[:, 2 * g:2 * g + 2, :], in0=s_ps[:],
                    in1=bc_rep.rearrange("p (x d) -> p x d", x=1).to_broadcast([128, 2, H]),
                    op=mybir.AluOpType.add)

            # ---------------- P3: edge phase -------------------------------
            ridx = cp.tile([128, T], I32)
            nc.sync.dma_start(out=ridx[:], in_=rlay_d[:])
            crel = cp.tile([128, T], F32)
            nc.sync.dma_start(out=crel[:], in_=crel_d[:])
            tagg = cp.tile([128, NWIN, 72], F32)

            MAXTW = int(tiles_w.max())
            for w in range(NWIN):
                tw = int(tiles_w[w])
                t0 = int(win_start[w])
                if tw == 0:
                    continue
                crelT_w = wp.tile([1, MAXTW * 128], F32, tag="crelTw")
                nc.sync.dma_start(out=crelT_w[0:1, :tw * 128],
                                  in_=crelT_d[0:1, t0 * 128:(t0 + tw) * 128])
                eaT_w = wp.tile([ED, MAXTW * 128], F32, tag="eaTw")
                nc.sync.dma_start(out=eaT_w[:, :tw * 128],
                                  in_=eaT_d[:, t0 * 128:(t0 + tw) * 128])
                hg = hgp.tile([128, MAXTW, 72], BF16, tag="hg")
                pw = ps.tile([128, 2 * MAXTW * H + 72], F32, tag="pswin")
                pay = hgp.tile([128, MAXTW, 72], BF16, tag="pay")
                al = wp.tile([128, MAXTW, H], F32, tag="al")
                srg = wp.tile([128, MAXTW, H], F32, tag="srg")
                o_tiles = wp.tile([128, MAXTW, 128], BF16, tag="otl")

                for tt in range(tw):
                    t = t0 + tt
                    nc.gpsimd.indirect_dma_start(
                        out=hg[:, tt, :], out_offset=None, in_=ttab_d[:],
                        in_offset=bass.IndirectOffsetOnAxis(ap=ridx[:, t:t + 1], axis=0))
                    # one-hot (edge-partition) and transposed one-hot
                    nc.vector.tensor_tensor(
                        out=o_tiles[:, tt, :],
                        in0=crel[:, t:t + 1].to_broadcast([128, 128]),
                        in1=iota_free[:], op=mybir.AluOpType.is_equal)
                    ct_ps = ps.tile([128, 128], F32, tag="ctps")
                    nc.tensor.matmul(out=ct_ps[:], lhsT=ones_row[:],
                                     rhs=crelT_w[0:1, tt * 128:(tt + 1) * 128],
                                     start=True, stop=True)
                    oT = wp.tile([128, 128], BF16, tag="oT")
                    nc.vector.tensor_tensor(
                        out=oT[:], in0=iota_col[:].to_broadcast([128, 128]),
                        in1=ct_ps[:], op=mybir.AluOpType.is_equal)
                    nc.tensor.matmul(out=pw[:, tt * H:(tt + 1) * H], lhsT=oT[:],
                                     rhs=sc_sb[:, w, :], start=True, stop=True)
                    nc.tensor.matmul(out=pw[:, (MAXTW + tt) * H:(MAXTW + tt + 1) * H],
                                     lhsT=eaT_w[:, tt * 128:(tt + 1) * 128],
                                     rhs=a_sb[:], start=True, stop=True)
                # alpha = scg + s_r + se ; lrelu ; exp
                nc.vector.tensor_copy(out=srg[:, :tw, :], in_=hg[:, :tw, 64:72])
                nc.vector.scalar_tensor_tensor(
                    out=al[:, :tw, :],
                    in0=pw[:, 0:tw * H].rearrange("p (t h) -> p t h", h=H),
                    scalar=0.0,
                    in1=srg[:, :tw, :], op0=mybir.AluOpType.add,
                    op1=mybir.AluOpType.add)
                nc.vector.tensor_tensor(
                    out=al[:, :tw, :], in0=al[:, :tw, :],
                    in1=pw[:, MAXTW * H:(MAXTW + tw) * H].rearrange(
                        "p (t h) -> p t h", h=H),
                    op=mybir.AluOpType.add)
                nc.scalar.activation(out=al[:, :tw, :], in_=al[:, :tw, :],
                                     func=mybir.ActivationFunctionType.Lrelu, alpha=0.01)
                nc.scalar.activation(out=pay[:, :tw, 64:72], in_=al[:, :tw, :],
                                     func=mybir.ActivationFunctionType.Exp)
                # msg = ex * h
                nc.vector.tensor_tensor(
                    out=pay[:, :tw, 0:64].rearrange("p t (h d) -> p t h d", h=H),
                    in0=hg[:, :tw, 0:64].rearrange("p t (h d) -> p t h d", h=H),
                    in1=pay[:, :tw, 64:72].rearrange(
                        "p t (h x) -> p t h x", x=1).to_broadcast([128, tw, H, HD]),
                    op=mybir.AluOpType.mult)
                # scatter: window accumulation in PSUM
                W0 = 2 * MAXTW * H
                for tt in range(tw):
                    nc.tensor.matmul(out=pw[:, W0:W0 + 72], lhsT=o_tiles[:, tt, :],
                                     rhs=pay[:, tt, :], start=(tt == 0),
                                     stop=(tt == tw - 1))
                nc.vector.tensor_copy(out=tagg[:, w, :], in_=pw[:, W0:W0 + 72])

            # ---------------- P4: normalize + output -----------------------
            den = wp.tile([128, NWIN, H], F32, tag="den")
            nc.vector.tensor_scalar(out=den[:], in0=tagg[:, :, 64:72],
                                    scalar1=1e-16, scalar2=None,
                                    op0=mybir.AluOpType.add)
            nc.vector.reciprocal(out=den[:], in_=den[:])
            nc.vector.tensor_tensor(
                out=tagg[:, :, 0:64].rearrange("p w (h d) -> p w h d", h=H),
                in0=tagg[:, :, 0:64].rearrange("p w (h d) -> p w h d", h=H),
                in1=den[:].rearrange("p w (h x) -> p w h x", x=1).to_broadcast(
                    [128, NWIN, H, HD]),
                op=mybir.AluOpType.mult)
            nc.vector.tensor_tensor(
                out=tagg[:, :, 0:64], in0=tagg[:, :, 0:64],
                in1=bias_rep.rearrange("p (x d) -> p x d", x=1).to_broadcast([128, NWIN, D]),
                op=mybir.AluOpType.add)
            for w in range(NWIN):
                xt_ps = ps.tile([D, 128], F32, tag="pbig")
                nc.tensor.transpose(out=xt_ps[:], in_=tagg[:, w, 0:64],
                                    identity=ident[:])
                xt = wp.tile([D, 128], F32R, tag="xt")
                nc.vector.tensor_copy(out=xt[:], in_=xt_ps[:])
                ow_ps = ps.tile([128, D], F32, tag="pmed")
                nc.tensor.matmul(out=ow_ps[:], lhsT=xt[:], rhs=woutT[:],
                                 start=True, stop=True)
                ow = wp.tile([128, D], F32, tag="ow")
                nc.vector.tensor_tensor(out=ow[:], in0=ow_ps[:], in1=bout_rep,
                                        op=mybir.AluOpType.add)
                nc.sync.dma_start(out=out_d[w * 128:(w + 1) * 128, :], in_=ow[:])

    _split_multi_waits(nc)
    return nc


# ---------------------------------------------------------------------------
_CACHE = {}
LAST_EXEC_NS = None


def kernel(feats, edge_index, edge_attr, W_fc, b_fc, W_edge, att, bias, W_out, b_out):
    feats_pad, cores, tiles_w, win_start, T = _host_prep(feats, edge_index, edge_attr)
    bd_c, bd_r, bd_e = _expand_att(att)

    key = ("v4", T, tuple(tiles_w.tolist()))
    if key not in _CACHE:
        _CACHE[key] = _build_nc(tiles_w, win_start, T)
    nc = _CACHE[key]

    shared = {
        "feats_pad": feats_pad,
        "W_fc": np.asarray(W_fc, np.float32),
        "b_fc_col": np.asarray(b_fc, np.float32).reshape(D, 1),
        "W_edge": np.asarray(W_edge, np.float32),
        "att_bd_c": bd_c, "att_bd_r": bd_r, "att_bd_e": bd_e,
        "bias_row": np.asarray(bias, np.float32).reshape(1, D),
        "W_out": np.asarray(W_out, np.float32),
        "b_out_row": np.asarray(b_out, np.float32).reshape(1, D),
    }
    in_maps = []
    for k in range(NCORES):
        m = dict(shared)
        m["feats_own"] = cores[k]["feats_own"]
        m["r_lay"] = cores[k]["r_lay"]
        m["crel_lay"] = cores[k]["crel_lay"]
        m["crelT_lay"] = cores[k]["crelT_lay"]
        m["eaT_lay"] = cores[k]["eaT_lay"]
        in_maps.append(m)

    trace = bool(int(os.environ.get("KERNEL_TRACE", "0")))
    res = run_bass_kernel_spmd(nc, in_maps, list(range(NCORES)), trace=trace)
    global LAST_EXEC_NS
    LAST_EXEC_NS = res.exec_time_ns
    out = np.concatenate(
        [res.results[k]["out_shard"][:NSH] for k in range(NCORES)], axis=0)
    return (out,
            np.asarray(edge_index),
            np.asarray(edge_attr, np.float32))
